# revision 34
# baseline (speedup 1.0000x reference)
"""Trainium2 Bass kernel for SimCLR NT-Xent contrastive loss (N=4096, D=512, T=0.5).

Math: with z = rownorm(concat(emb_i, emb_j)) (8192x512) and S = z @ z.T:
  denom_r = rowsum_r(exp(S/T)) - e^2;  loss = mean_r[log denom_r - 2*S[r, r+N mod 2N]]

Distribution (symmetric, data-parallel): 8 blocks of 1024 rows. Core c gets the
5120-row window starting at row 1024c (host np.roll slice) and computes
exp-blocks against its own block L0 = rows [0:1024):
  - (L0, L1..L3): full 1024x1024 blocks -> row-sums (denom partials of L0 rows,
    via ACT exp accum) and column-sums (denom partials of L1..L3 rows, via
    fp8 DoubleRow ones-matmul over stacked m-tile pairs of exp values).
  - (L0, L0) and (L0, L4): "banded" blocks over 128-col tiles at tile distance
    dt = (jt - it) mod 8 in {0..4}: rows summed at full weight for dt 0..4;
    col-sums only for dt 1..3.  Summed over all 8 cores this counts every
    unordered pair exactly once: dt in {1,2,3} / distance in {1,2,3} pairs are
    computed by exactly one core (row+col), while the self-paired classes
    (dt=0 tiles and the dt=4 / distance-4 classes) are computed by both
    involved cores, each contributing its own rows' sums.
Host merges per-row partial denominators, subtracts e^2, takes log, adds the
positive-pair terms (per-row dots computed on-device).

Numerics: z stored as fp8e4 scaled by 64 (dots = 4096*sim; exp scale 2^-11).
Matmuls run fp8 DoubleRow (2 contraction planes of 128 -> 2x PE throughput);
zT built by a u16 DMA-transpose of byte-paired fp8 + a deinterleave copy pass
into plane-major layout (DoubleRow requires >=16B plane stride).
"""

import numpy as np

for _p in ("/opt/trn_rl_repo", "/root/.axon_site/_ro/trn_rl_repo"):
    try:
        import concourse  # noqa: F401
        break
    except ImportError:
        import sys
        if _p not in sys.path:
            sys.path.insert(0, _p)

import concourse.bass as bass
import concourse.bacc as bacc
import concourse.tile as tile
from concourse import mybir
from concourse.bass_utils import run_bass_kernel_spmd

F32 = mybir.dt.float32
I32 = mybir.dt.int32
U16 = mybir.dt.uint16
BF16 = mybir.dt.bfloat16
FP8 = mybir.dt.float8e4
ALU = mybir.AluOpType
AF = mybir.ActivationFunctionType
DR = mybir.MatmulPerfMode.DoubleRow

N_CORES = 8
BATCH = 4096
DIM = 512
ROWS = 2 * BATCH
BLOCK = 1024
NBLK = 5
WROWS = NBLK * BLOCK
P = 128
MT = BLOCK // P             # 8 m-tiles per block
BAND = 5 * P                # banded width per m-tile (dt 0..4)
MAGIC = 0x5F3759DF
EXPSC = 2.0 / 4096.0
N_DUMMY = 40                # PE warmup matmuls


def _band_pieces(m):
    """Banded m-tile: map band-relative psum cols [0:640) to zT cols
    [(128m + q) mod 1024], split at the mod-1024 wrap AND at psum col 512
    (matmul output must stay within one 2KB PSUM bank).
    Returns [(dst0, src0, length), ...]."""
    pieces = []
    dst = 0
    while dst < BAND:
        src = (m * P + dst) % BLOCK
        lim = min(BAND - dst, BLOCK - src)
        if dst < 512:
            lim = min(lim, 512 - dst)
        pieces.append((dst, src, lim))
        dst += lim
    return pieces


def _build_program():
    nc = bacc.Bacc(trn_type="TRN2")
    x_in = nc.declare_dram_parameter("x", [WROWS, DIM], F32, isOutput=False)
    drow_out = nc.declare_dram_parameter("drow", [P, MT], F32, isOutput=True)
    dcol_out = nc.declare_dram_parameter("dcol", [1, NBLK * BLOCK], F32,
                                         isOutput=True)
    pos_out = nc.declare_dram_parameter("pos", [P, MT], F32, isOutput=True)

    with tile.TileContext(nc) as tc:
        with tc.tile_pool(name="xg", bufs=3) as xg_pool, \
             tc.tile_pool(name="small", bufs=2) as small_pool, \
             tc.tile_pool(name="sq", bufs=2) as sq_pool, \
             tc.tile_pool(name="zt", bufs=4) as zt_pool, \
             tc.tile_pool(name="es", bufs=8) as e_pool, \
             tc.tile_pool(name="single", bufs=1) as singles, \
             tc.tile_pool(name="zdram", bufs=1, space="DRAM") as dram_pool, \
             tc.tile_pool(name="psum", bufs=3, space="PSUM") as psum_pool, \
             tc.tile_pool(name="cps", bufs=1, space="PSUM") as cps_pool:

            n2 = singles.tile([P, NBLK * MT], F32, tag="n2")
            inv = singles.tile([P, NBLK * MT], F32, tag="inv")
            magic8 = singles.tile([P, MT], I32, tag="magic8")
            nc.vector.memset(magic8, MAGIC)
            accm = singles.tile([P, MT, 6], F32, tag="accm")
            nc.vector.memset(accm.rearrange("p a b -> p (a b)"), 0.0)
            pos_acc = singles.tile([P, MT], F32, tag="pos_acc")
            onesf = singles.tile([P, 16], FP8, tag="onesf")
            nc.vector.memset(onesf, 1.0)
            onesf2 = singles.tile([P, 2, 16], FP8, tag="onesf2")
            nc.vector.memset(onesf2.rearrange("p a b -> p (a b)"), 1.0)
            onesb = singles.tile([P, 16], BF16, tag="onesb")
            nc.vector.memset(onesb, 1.0)
            drhs = singles.tile([P, 256], BF16, tag="drhs")
            nc.vector.memset(drhs, 0.0)
            colout = singles.tile([1, NBLK * BLOCK], F32, tag="colout")

            zblk = [singles.tile([P, MT, DIM], FP8, tag=f"zb{b}", name=f"zb{b}")
                    for b in range(NBLK)]
            zTs = [[singles.tile([P, 2, BLOCK], FP8, tag=f"zs{b}_{k}",
                                 name=f"zs{b}_{k}") for k in range(2)]
                   for b in range(NBLK)]
            zd = [dram_pool.tile([BLOCK, DIM], FP8, tag=f"zd{b}", name=f"zd{b}")
                  for b in range(NBLK)]

            # ---- PE warmup dummies (keep HAM busy through the load phase) ----
            dps = psum_pool.tile([P, BLOCK], F32, tag="ps", name="dps")
            for i in range(N_DUMMY):
                nc.tensor.matmul(dps[0:16, 0:256], lhsT=onesb[:, 0:16],
                                 rhs=drhs, start=True, stop=True,
                                 skip_group_check=True)

            colps = {}
            e_tiles = {}
            xg_tiles = {}

            def emit_load(b):
                if b >= NBLK or b in xg_tiles:
                    return
                xg = xg_pool.tile([P, MT, DIM], F32, tag="xg", name=f"xg{b}")
                xg_tiles[b] = xg
                nc.sync.dma_start(
                    out=xg,
                    in_=x_in[b * BLOCK:(b + 1) * BLOCK, :].rearrange(
                        "(a p) d -> p a d", p=P))

            emit_load(0)
            emit_load(1)

            def norm_block(b):
                """rownorm 1024 rows, cast fp8*64, write zd, transpose,
                deinterleave into zTs[b]. (load prefetched one block ahead)"""
                xg = xg_tiles.pop(b)
                for a in range(MT):
                    sq = sq_pool.tile([P, DIM], F32, tag="sq")
                    # accum of (x * 2^-12) * x = ||x||^2/4096 per row
                    nc.vector.scalar_tensor_tensor(
                        out=sq, in0=xg[:, a, :], scalar=1.0 / 4096.0,
                        in1=xg[:, a, :], op0=ALU.mult, op1=ALU.mult,
                        accum_out=n2[:, b * MT + a: b * MT + a + 1])
                # rsqrt via Quake seed + 2 Newton steps -> 64/||x||
                sl = n2[:, b * MT:(b + 1) * MT]
                isl = inv[:, b * MT:(b + 1) * MT]
                sh = small_pool.tile([P, MT], I32, tag="sh")
                nc.vector.tensor_scalar(
                    out=sh, in0=sl.bitcast(I32), scalar1=1, scalar2=None,
                    op0=ALU.logical_shift_right)
                seed = small_pool.tile([P, MT], I32, tag="seed")
                nc.vector.scalar_tensor_tensor(
                    out=seed, in0=magic8, scalar=0.0, in1=sh,
                    op0=ALU.bypass, op1=ALU.subtract)
                y = seed.bitcast(F32)
                for it in range(2):
                    ta = small_pool.tile([P, MT], F32, tag="ta")
                    tb = small_pool.tile([P, MT], F32, tag="tb")
                    nc.vector.tensor_mul(out=ta, in0=y, in1=y)
                    nc.vector.scalar_tensor_tensor(
                        out=tb, in0=ta, scalar=-0.5, in1=sl,
                        op0=ALU.mult, op1=ALU.mult)
                    nc.vector.tensor_scalar(
                        out=tb, in0=tb, scalar1=1.5, scalar2=None, op0=ALU.add)
                    dst = isl if it == 1 else y
                    nc.vector.tensor_mul(out=dst, in0=y, in1=tb)
                for a in range(MT):
                    nc.vector.tensor_scalar_mul(
                        out=zblk[b][:, a, :], in0=xg[:, a, :],
                        scalar1=inv[:, b * MT + a: b * MT + a + 1])
                nc.sync.dma_start(
                    out=zd[b][:, :].rearrange("(s p) d -> p s d", p=P),
                    in_=zblk[b])
                zdu = zd[b].bitcast(U16)  # [1024, 256]
                for kp in range(2):
                    zt = zt_pool.tile([P, BLOCK], U16, tag="zt")
                    nc.sync.dma_start_transpose(
                        out=zt, in_=zdu[:, kp * P:(kp + 1) * P])
                    ztf = zt.bitcast(FP8).rearrange("p (r two) -> p two r",
                                                    two=2)
                    for j in range(2):
                        dst = zTs[b][kp][:, j, :]
                        if (kp + j) % 2 == 0:
                            nc.vector.tensor_scalar(
                                out=dst, in0=ztf[:, j, :],
                                scalar1=1.0, scalar2=None, op0=ALU.mult)
                        else:
                            nc.scalar.copy(out=dst, in_=ztf[:, j, :])

            def mains(b):
                """matmul + exp row-sums for block pair (L0, Lb)."""
                banded = b in (0, 4)
                slot = {0: 0, 4: 1, 1: 2, 2: 3, 3: 4}[b]
                for m in range(MT):
                    ps = psum_pool.tile([P, BLOCK], F32, tag="ps")
                    if banded:
                        # band-relative: psum cols [0:640) = zT cols
                        # (128m + q) mod 1024
                        for (dst, src, ln) in _band_pieces(m):
                            for kp in range(2):
                                nc.tensor.matmul(
                                    ps[:, dst:dst + ln],
                                    lhsT=zTs[0][kp][:, :, m * P:(m + 1) * P],
                                    rhs=zTs[b][kp][:, :, src:src + ln],
                                    start=(kp == 0), stop=(kp == 1),
                                    perf_mode=DR)
                        et = e_pool.tile([P, BAND], FP8, tag="eb")
                        e_tiles[(b, m)] = et
                        nc.scalar.activation(
                            out=et, in_=ps[:, 0:BAND], func=AF.Exp,
                            scale=EXPSC,
                            accum_out=accm[:, m, slot:slot + 1])
                    else:
                        for half in range(2):
                            for kp in range(2):
                                nc.tensor.matmul(
                                    ps[:, half * 512:(half + 1) * 512],
                                    lhsT=zTs[0][kp][:, :, m * P:(m + 1) * P],
                                    rhs=zTs[b][kp][:, :, half * 512:
                                                   (half + 1) * 512],
                                    start=(kp == 0), stop=(kp == 1),
                                    perf_mode=DR)
                        if m % 2 == 0:
                            et = e_pool.tile([P, 2, BLOCK], FP8, tag="ep")
                            e_tiles[(b, m // 2)] = et
                        else:
                            et = e_tiles[(b, m // 2)]
                        nc.scalar.activation(
                            out=et[:, m % 2, :], in_=ps, func=AF.Exp,
                            scale=EXPSC,
                            accum_out=accm[:, m, slot:slot + 1])

            def cols_start(b):
                cp = cps_pool.tile([1, BLOCK], F32, tag="cp", name=f"cp{b}")
                colps[b] = cp

            def cols_banded_m(b, m):
                """banded col-sums dt 1..3; one PSUM start/stop per 2KB bank."""
                cp = colps[b]
                et = e_tiles[(b, m)]
                for dt in (1, 2, 3):
                    jc = (m + dt) % MT
                    bank = jc // 4
                    st = (m, dt) == ((0, 1) if bank == 0 else (1, 3))
                    sp = (m, dt) == ((7, 3) if bank == 0 else (6, 1))
                    nc.tensor.matmul(
                        cp[0:1, jc * P:(jc + 1) * P],
                        lhsT=onesf[:, 0:1],
                        rhs=et[:, dt * P:(dt + 1) * P],
                        start=st, stop=sp, skip_group_check=True)

            def cols_full_mp(b, mp):
                """full-block col-sums: DR ones-matmul over an m-tile pair."""
                cp = colps[b]
                et = e_tiles[(b, mp)]
                for half in range(2):
                    nc.tensor.matmul(
                        cp[0:1, half * 512:(half + 1) * 512],
                        lhsT=onesf2[:, :, 0:1],
                        rhs=et[:, :, half * 512:(half + 1) * 512],
                        start=(mp == 0), stop=(mp == MT // 2 - 1),
                        perf_mode=DR, skip_group_check=True)

            def cols_finish(b):
                cp = colps.pop(b)
                nc.vector.tensor_scalar(
                    out=colout[0:1, b * BLOCK:(b + 1) * BLOCK],
                    in0=cp, scalar1=1.0, scalar2=None, op0=ALU.mult)
                nm = MT if b in (0, 4) else MT // 2
                for k in range(nm):
                    e_tiles.pop((b, k), None)

            def emit_cols(b):
                cols_start(b)
                if b in (0, 4):
                    for m in range(MT):
                        cols_banded_m(b, m)
                else:
                    for mp in range(MT // 2):
                        cols_full_mp(b, mp)
                cols_finish(b)

            # ---- emission: per-block pipeline, MM phase lagging one block ----
            for b in range(NBLK):
                norm_block(b)
                emit_load(b + 2)
                if b >= 1:
                    mains(b - 1)
                    emit_cols(b - 1)
            # pos dots: z_L0[i] . z_L4[i] (raw, x4096 scale)
            for s in range(MT):
                psc = sq_pool.tile([P, DIM], BF16, tag="psc")
                nc.vector.scalar_tensor_tensor(
                    out=psc, in0=zblk[0][:, s, :], scalar=0.0,
                    in1=zblk[4][:, s, :], op0=ALU.bypass, op1=ALU.mult,
                    accum_out=pos_acc[:, s:s + 1])
            mains(4)
            emit_cols(4)

            # ---- outputs ----
            drow = singles.tile([P, MT], F32, tag="drow")
            for m in range(MT):
                nc.vector.reduce_sum(
                    out=drow[:, m:m + 1], in_=accm[:, m, :],
                    axis=mybir.AxisListType.X)
            nc.sync.dma_start(out=drow_out[:, :], in_=drow)
            nc.sync.dma_start(out=dcol_out[:, :], in_=colout)
            nc.sync.dma_start(out=pos_out[:, :], in_=pos_acc)

    nc.finalize()
    return nc


_CACHE = {}


def _run(full: np.ndarray, trace: bool = False, **kwargs):
    if "nc" not in _CACHE:
        _CACHE["nc"] = _build_program()
    nc = _CACHE["nc"]
    in_maps = []
    for c in range(N_CORES):
        idx0 = (c * BLOCK) % ROWS
        win = np.concatenate([full[idx0:], full[:idx0]], axis=0)[:WROWS]
        in_maps.append({"x": np.ascontiguousarray(win)})
    return run_bass_kernel_spmd(
        nc, in_maps, core_ids=list(range(N_CORES)), trace=trace, **kwargs)


def _merge(results) -> np.ndarray:
    den = np.zeros(ROWS, dtype=np.float64)
    pos = np.zeros(ROWS, dtype=np.float64)
    for c, r in enumerate(results):
        rows0 = np.arange(BLOCK) + BLOCK * c
        den[rows0] += r["drow"].astype(np.float64).T.reshape(-1)
        pos[rows0] = r["pos"].astype(np.float64).T.reshape(-1) / 4096.0
        dcol = r["dcol"].astype(np.float64).reshape(NBLK, BLOCK)
        for j in range(NBLK):
            rows_j = (np.arange(BLOCK) + BLOCK * ((c + j) % N_CORES)) % ROWS
            den[rows_j] += dcol[j]
    denom = den - np.exp(2.0)
    loss = np.mean(np.log(denom) - 2.0 * pos)
    return np.array(loss, dtype=np.float32)


def kernel(emb_i: np.ndarray, emb_j: np.ndarray) -> np.ndarray:
    full = np.concatenate(
        [np.asarray(emb_i, np.float32), np.asarray(emb_j, np.float32)], axis=0)
    return _merge(_run(full).results)
